# revision 29
# baseline (speedup 1.0000x reference)
"""ObjectAttentionBlock2D TRN2 kernel (v2: transposed-output pipeline).

Reference computation (per batch b):
    xf    = x[b].reshape(C, N)                  # C=512, N=128*128=16384
    pf    = proxy[b,:,:,0]                      # [C, K], K=64
    query = Wq @ xf + bq                        # [Ck=256, N]
    keym  = Wk @ pf + bk                        # [Ck, K]
    value = (Wv @ pf + bv).T                    # [K, Cv=256]
    sim   = softmax_k(query.T @ keym / 16)      # [N, K]
    ctx   = sim @ value                         # [N, Cv]
    out   = Wo @ ctx.T + bo                     # [C, N]

Sharding: data-parallel over batch. B=8 images -> 8 NeuronCores, no
collectives. Weights replicated.

Algebra: all four projections fold into small per-image matrices computed
on-device from pf and HOST-FOLDED weight products (weight-only folding:
wqk = Wk^T Wq, wovT = (Wo Wv)^T, wqbk = Wq^T bk, wkbq = Wk^T(bq/16),
wobvo = Wo bv + bo -- none touch input data):
  msim  = wqk^T pf + wqbk 1^T   [C, K]   simT = msim^T x
  sbias = pf^T wkbq             [K, 1]   rides in exp's bias slot (the
                                         k-independent bq.bk/16 term is
                                         dropped: softmax cancels it)
  wvt   = pf^T wovT + 1 wobvo^T [K, C]   out = wvt^T en (value bias and +bo
                                         fold in: softmax cols sum to 1)

The second matmul runs TRANSPOSED (pixels on PSUM partitions):
outT[n, c] = e[:, n]^T wvt with lhsT = e sub-tile [K=64, 128 px]. The
softmax denominator then lands as a per-partition column (denT = e^T ones
via a 1-column matmul, ~free in PE time), so the reciprocal folds into the
PSUM->SBUF fp16 downcast as the per-partition `scale` operand of ACT/DVE
ops -- no broadcast matmul, no separate normalize multiply.

Output is written as OUT_T [N, C] fp16 (host transposes + upcasts),
halving the dominant out DMA stream vs f32. x is split channel-chunk-wise:
first (4-N8) chunks of 128 channels in fp16, last N8 chunks in fp8 E3M4
(4-bit mantissa, matches TRN FP8_EXP3); N8 trades DMA bytes vs accuracy:
N8=0 -> 103.1us / 7.9e-4, N8=2 -> ~85us / 1.26e-2, N8=4 -> 76.9us /
1.71e-2 (gate 2e-2; deterministic, dominated by x quantization).

Per 512-px period (32 periods/core, DMA floor 1092ns at N8=4):
  DMA in: x8 [128,N8,512] f8 (+ x16 f16)    512B+ contiguous elements
  PE: 4 full-width sim MMs [64,512] (PE.SEQ cost is per-instruction) ->
      4 denT MMs (out free 1) -> 4 outT MMs (lhsT=e [64,128], rhs=wvt)
  ACT: one exp over [64,512] (bias=sbias, scale=1/16) -> e fp16;
       2 (or 1 every 3rd period) downcasts (Identity, scale=recipT col)
  DVE: reciprocal [128,4]; remaining downcasts (tensor_scalar mult)
  DMA out: one [128,4,512] fp16 -> OUT_T via Pool SWDGE (its wait-on-casts
      must not block SP's x prefetch run-ahead; last 3 via idle SP HWDGE)
Schedule: the whole x stream is front-loaded (32-deep SBUF ring, SP queue
runs ahead freely), so the trailing DMA work is outs whose compute finished
long ago -- DMA engines run 100% busy from ~2.3us to ~75.4us. The tail
stage (den/recip/outMM/cast/outDMA) lags one period so PE never stalls on
exp; psum: 2 sim banks + 6-deep shared out/den ring. A dummy early matmul
warms the cost model's PE p-state ramp so setup MMs run at full clock.
"""

from collections import deque

import numpy as np
import ml_dtypes

import concourse.bacc as bacc
import concourse.mybir as mybir
import concourse.tile as tile
from concourse import bass_utils

F32 = mybir.dt.float32
F32R = mybir.dt.float32r
F16 = mybir.dt.float16
F8 = mybir.dt.float8e3

B, C, H, W = 8, 512, 128, 128
N = H * W                    # 16384 pixels per image
CK, CV, K = 256, 256, 64
P = 128                      # SBUF partitions
FP = 512                     # pixels per period (DMA granule)
NT = 4                       # sub-tiles per period
FT = FP // NT                # 128 pixels per sub-tile (= out PSUM partitions)
NP = N // FP                 # 32 periods
CI_CH = C // P               # 4 contraction chunks over C
Q_CH = CK // P               # 2 chunks over Ck
V_CH = CV // P               # 2 chunks over Cv
SCALE = CK ** -0.5           # 1/16
PF = 24                      # x prefetch run-ahead (SP queue depth paces actual issue)

N8 = 0                       # trailing C-chunks of x stored as fp8 E3M4
C16 = CI_CH - N8

_CACHED = None


def _build(n8):
    c16 = CI_CH - n8
    nc = bacc.Bacc("TRN2", target_bir_lowering=False, debug=False)

    X16 = (
        nc.dram_tensor("x16", [c16 * P, N], F16, kind="ExternalInput").ap()
        if c16 else None
    )
    X8 = (
        nc.dram_tensor("x8", [n8 * P, N], F8, kind="ExternalInput").ap()
        if n8 else None
    )
    # host-folded weight products (weights only, no input data):
    # wqk[c', c] = (Wk^T Wq)[c', c];  wovT[c', c] = (Wo Wv)^T[c', c]
    PF_T = nc.dram_tensor("pf16", [C, K], F16, kind="ExternalInput").ap()
    WQK = nc.dram_tensor("wqk", [C, C], F16, kind="ExternalInput").ap()
    WOVT = nc.dram_tensor("wovT", [C, C], F16, kind="ExternalInput").ap()
    # crow = [ones(64) | wobvo = Wo@bv+bo (512) | wqbk = Wq^T bk (512)]
    # (the SCALE*bq.bk logit constant is k-independent, softmax cancels it)
    CROW = nc.dram_tensor("crow", [1, 1088], F32, kind="ExternalInput").ap()
    # wkbq[c'] = Wk^T @ (bq*SCALE), chunked per-partition
    WKBQ = nc.dram_tensor("wkbq", [P, CI_CH], F16, kind="ExternalInput").ap()
    OUTT = nc.dram_tensor("outt", [N, C], F16, kind="ExternalOutput").ap()

    x16_r = X16.rearrange("(co p) n -> p co n", p=P) if c16 else None
    x8_r = X8.rearrange("(co p) n -> p co n", p=P) if n8 else None
    outt_r = OUTT.rearrange("(q t p) c -> p q t c", t=NT, p=P)

    Exp = mybir.ActivationFunctionType.Exp
    Ident = mybir.ActivationFunctionType.Identity

    with tile.TileContext(nc) as tc:
        with tc.tile_pool(name="const", bufs=1) as cp:
            pf_t = cp.tile([P, CI_CH, K], F16)
            nc.sync.dma_start(pf_t, PF_T.rearrange("(co p) k -> p co k", p=P))
            wqk = cp.tile([P, CI_CH, C], F16)
            nc.sync.dma_start(wqk, WQK.rearrange("(co p) c -> p co c", p=P))
            wovt = cp.tile([P, CI_CH, C], F16)
            # Pool SWDGE: lands concurrently without delaying x on SP/HWDGE
            nc.gpsimd.dma_start(wovt, WOVT.rearrange("(co p) c -> p co c", p=P))
            crow = cp.tile([1, 1088], F32R)
            nc.scalar.dma_start(crow, CROW.bitcast(F32R))
            ones_row = crow[:, 0:K]
            wobvo_row = crow[:, K:K + C]
            wqbk_row = crow[:, K + C:K + C + C]
            wkbq = cp.tile([P, CI_CH], F16)
            nc.scalar.dma_start(wkbq, WKBQ)
            ones_col = cp.tile([K, 1], F16)
            nc.vector.memset(ones_col, 1.0)

            wvt = cp.tile([K, C], F16)           # (Wo @ value^T)^T + bo
            msim = cp.tile([P, CI_CH, K], F16)   # M[c,k] = (Wq^T keym)[c,k]
            sbias = cp.tile([K, 1], F32)         # (bq/16)^T keym

            # ---- one-time, all straight from pf via host-folded weights
            with tc.tile_pool(name="setup_ps", bufs=1, space="PSUM") as sps:
                # dummy matmul as early as possible: the cost model's PE
                # p-state ramp counts from the first continuous-busy start,
                # so warming here gets setup MMs toward full clock early
                warm = sps.tile([1, 1], F32)
                nc.tensor.matmul(warm, ones_col, ones_col, start=True, stop=True)
                # msim[c,k] = sum_c' wqk[c',c] pf[c',k] + (Wq^T bk)[c] 1[k]
                mps = sps.tile([P, CI_CH, K], F32)
                for ci in range(CI_CH):
                    for cj in range(CI_CH):
                        nc.tensor.matmul(
                            mps[:, ci, :],
                            wqk[:, cj, ci * P:(ci + 1) * P],
                            pf_t[:, cj, :],
                            start=(cj == 0), stop=False,
                        )
                    nc.tensor.matmul(
                        mps[:, ci, :],
                        wqbk_row[:, ci * P:(ci + 1) * P],
                        ones_row,
                        start=False, stop=True,
                    )
                nc.vector.tensor_copy(msim, mps)

                sbps = sps.tile([K, 1], F32)
                for cj in range(CI_CH):
                    nc.tensor.matmul(
                        sbps, pf_t[:, cj, :], wkbq[:, cj:cj + 1],
                        start=(cj == 0), stop=(cj == CI_CH - 1),
                    )
                nc.vector.tensor_copy(sbias, sbps)

                wvtps = sps.tile([K, C], F32)
                for cj in range(CI_CH):
                    nc.tensor.matmul(
                        wvtps, pf_t[:, cj, :], wovt[:, cj, :],
                        start=(cj == 0), stop=False,
                    )
                # += ones[k] * (Wo bv + bo)[c]: softmax columns sum to 1, so
                # a constant row realises both the value bias and +bo.
                nc.tensor.matmul(
                    wvtps, ones_row, wobvo_row,
                    start=False, stop=True,
                )
                nc.vector.tensor_copy(wvt, wvtps)

            # ---- steady-state software pipeline over 512-px periods
            with (
                tc.tile_pool(name="xin16", bufs=(32 if c16 <= 2 else 26)) as xp16,
                tc.tile_pool(name="xin8", bufs=32) as xp8,
                tc.tile_pool(name="esb", bufs=8) as ep,
                tc.tile_pool(name="rsb", bufs=8) as rp,
                tc.tile_pool(name="outsb", bufs=16) as outp,
                tc.tile_pool(name="simps", bufs=2, space="PSUM") as simps,
                tc.tile_pool(name="outps", bufs=6, space="PSUM") as outps,
            ):
                xq = deque()

                def dispatch_x(j):
                    if j >= NP:
                        return
                    n0 = j * FP
                    t16 = t8 = None
                    if c16:
                        t16 = xp16.tile([P, c16, FP], F16, tag="x16")
                        nc.sync.dma_start(t16, x16_r[:, :, n0:n0 + FP])
                    if n8:
                        t8 = xp8.tile([P, n8, FP], F8, tag="x8")
                        nc.sync.dma_start(t8, x8_r[:, :, n0:n0 + FP])
                    xq.append((t16, t8))

                def tail(e_t, j):
                    den = outps.tile([P, NT], F32, tag="ops")
                    for t in range(NT):
                        nc.tensor.matmul(
                            den[:, t:t + 1], e_t[:, t * FT:(t + 1) * FT], ones_col,
                            start=True, stop=True,
                        )
                    r = rp.tile([P, NT], F32, tag="r")
                    with nc.allow_low_precision(reason="positive softmax denom"):
                        nc.vector.reciprocal(r, den)
                    o_sb = outp.tile([P, NT, C], F16, tag="osb")
                    for t in range(NT):
                        ops = outps.tile([P, C], F32, tag="ops")
                        nc.tensor.matmul(
                            ops, e_t[:, t * FT:(t + 1) * FT], wvt, start=True, stop=True,
                        )
                        sc = r[:, t:t + 1]
                        # alternate 2/2 and 1/3 ACT/DVE cast split so neither
                        # engine's steady load exceeds ~70% of the DMA period
                        n_act = 2 if (j % 3 != 2) else 1
                        if t < n_act:
                            nc.scalar.activation(o_sb[:, t, :], ops, Ident, scale=sc)
                        else:
                            nc.vector.tensor_scalar_mul(o_sb[:, t, :], ops, sc)
                    # out rides the Pool SWDGE queue: its wait-on-casts must
                    # not block SP's run-ahead x prefetch dispatches. In the
                    # drain (x stream done) SP is free and HWDGE has lower
                    # dispatch latency, so the last outs go there.
                    if j >= NP - 3:
                        nc.sync.dma_start(outt_r[:, j, :, :], o_sb)
                    else:
                        nc.gpsimd.dma_start(outt_r[:, j, :, :], o_sb)

                for j in range(PF):
                    dispatch_x(j)
                pend = deque()
                for ip in range(NP):
                    x16_t, x8_t = xq.popleft()
                    # full-width sim MMs: 4 instructions/period (PE.SEQ
                    # dispatch cost is per-instruction, engine time is not)
                    sim = simps.tile([K, FP], F32, tag="sim")
                    for ci in range(CI_CH):
                        if ci < c16:
                            src = x16_t[:, ci, :]
                        else:
                            src = x8_t[:, ci - c16, :]
                        nc.tensor.matmul(
                            sim, msim[:, ci, :], src,
                            start=(ci == 0), stop=(ci == CI_CH - 1),
                        )
                    e_t = ep.tile([K, FP], F16, tag="e")
                    nc.scalar.activation(e_t, sim, Exp, scale=SCALE, bias=sbias)
                    dispatch_x(ip + PF)
                    if pend:
                        tail(*pend.popleft())
                    pend.append((e_t, ip))
                while pend:
                    tail(*pend.popleft())

    nc.compile()
    return nc


def _get_nc():
    global _CACHED
    if _CACHED is None:
        _CACHED = _build(N8)
    return _CACHED


def kernel(x, proxy, Wq, bq, Wk, bk, Wv, bv, Wo, bo, **run_kwargs):
    nc = _get_nc()

    f32 = np.float32
    Wqf, Wkf, Wvf, Wof = (np.asarray(a, f32) for a in (Wq, Wk, Wv, Wo))
    bqf, bkf, bvf, bof = (np.asarray(a, f32) for a in (bq, bk, bv, bo))
    wqk = (Wkf.astype(np.float64).T @ Wqf.astype(np.float64)).astype(f32)
    wovT = (Wof.astype(np.float64) @ Wvf.astype(np.float64)).T.astype(f32)
    wobvo = Wof @ bvf + bof
    wqbk = Wqf.T @ bkf
    wkbq = (Wkf.T @ (bqf * np.float32(SCALE))).reshape(CI_CH, P).T
    crow = np.concatenate(
        [np.ones((1, K), f32), wobvo.reshape(1, C),
         wqbk.reshape(1, C)], axis=1)
    shared = {
        "wqk": np.ascontiguousarray(wqk).astype(np.float16),
        "wovT": np.ascontiguousarray(wovT).astype(np.float16),
        "wkbq": np.ascontiguousarray(wkbq).astype(np.float16),
        "crow": np.ascontiguousarray(crow),
    }
    in_maps = []
    for b in range(B):
        m = dict(shared)
        xf = np.asarray(x[b]).reshape(C, N)
        if C16:
            m["x16"] = np.ascontiguousarray(xf[: C16 * P]).astype(np.float16)
        if N8:
            m["x8"] = np.ascontiguousarray(xf[C16 * P:]).astype(
                ml_dtypes.float8_e3m4
            )
        m["pf16"] = np.ascontiguousarray(
            np.asarray(proxy[b, :, :, 0])
        ).astype(np.float16)
        in_maps.append(m)
    res = bass_utils.run_bass_kernel_spmd(
        nc, in_maps, core_ids=list(range(B)), **run_kwargs
    )
    out = np.stack(
        [
            np.asarray(res.results[b]["outt"]).astype(np.float32).T
            for b in range(B)
        ],
        axis=0,
    )
    if run_kwargs:
        kernel.last_results = res
    return out.reshape(B, C, H, W)


# revision 30
# speedup vs baseline: 1.3030x; 1.3030x over previous
"""ObjectAttentionBlock2D TRN2 kernel (v2: transposed-output pipeline).

Reference computation (per batch b):
    xf    = x[b].reshape(C, N)                  # C=512, N=128*128=16384
    pf    = proxy[b,:,:,0]                      # [C, K], K=64
    query = Wq @ xf + bq                        # [Ck=256, N]
    keym  = Wk @ pf + bk                        # [Ck, K]
    value = (Wv @ pf + bv).T                    # [K, Cv=256]
    sim   = softmax_k(query.T @ keym / 16)      # [N, K]
    ctx   = sim @ value                         # [N, Cv]
    out   = Wo @ ctx.T + bo                     # [C, N]

Sharding: data-parallel over batch. B=8 images -> 8 NeuronCores, no
collectives. Weights replicated.

Algebra: all four projections fold into small per-image matrices computed
on-device from pf and HOST-FOLDED weight products (weight-only folding:
wqk = Wk^T Wq, wovT = (Wo Wv)^T, wqbk = Wq^T bk, wkbq = Wk^T(bq/16),
wobvo = Wo bv + bo -- none touch input data):
  msim  = wqk^T pf + wqbk 1^T   [C, K]   simT = msim^T x
  sbias = pf^T wkbq             [K, 1]   rides in exp's bias slot (the
                                         k-independent bq.bk/16 term is
                                         dropped: softmax cancels it)
  wvt   = pf^T wovT + 1 wobvo^T [K, C]   out = wvt^T en (value bias and +bo
                                         fold in: softmax cols sum to 1)

The second matmul runs TRANSPOSED (pixels on PSUM partitions):
outT[n, c] = e[:, n]^T wvt with lhsT = e sub-tile [K=64, 128 px]. The
softmax denominator then lands as a per-partition column (denT = e^T ones
via a 1-column matmul, ~free in PE time), so the reciprocal folds into the
PSUM->SBUF fp16 downcast as the per-partition `scale` operand of ACT/DVE
ops -- no broadcast matmul, no separate normalize multiply.

Output is written as OUT_T [N, C] fp16 (host transposes + upcasts),
halving the dominant out DMA stream vs f32. x is split channel-chunk-wise:
first (4-N8) chunks of 128 channels in fp16, last N8 chunks in fp8 E3M4
(4-bit mantissa, matches TRN FP8_EXP3); N8 trades DMA bytes vs accuracy:
N8=0 -> 100.2us / 7.1e-4, N8=2 -> 89.0us / 1.22e-2, N8=4 -> 76.9us /
1.71e-2 (gate 2e-2; deterministic, dominated by x quantization). All three
sit on their DMA floor: boot ~2.0us + bytes/360GBps + drain ~1.7us.

Per 512-px period (32 periods/core, DMA floor 1092ns at N8=4):
  DMA in: x8 [128,N8,512] f8 (+ x16 f16)    512B+ contiguous elements
  PE: 4 full-width sim MMs [64,512] (PE.SEQ cost is per-instruction) ->
      4 denT MMs (out free 1) -> 4 outT MMs (lhsT=e [64,128], rhs=wvt)
  ACT: one exp over [64,512] (bias=sbias, scale=1/16) -> e fp16;
       2 (or 1 every 3rd period) downcasts (Identity, scale=recipT col)
  DVE: reciprocal [128,4]; remaining downcasts (tensor_scalar mult)
  DMA out: one [128,4,512] fp16 -> OUT_T via Pool SWDGE (its wait-on-casts
      must not block SP's x prefetch run-ahead; last 3 via idle SP HWDGE)
Schedule: the whole x stream is front-loaded (32-deep SBUF ring, SP queue
runs ahead freely), so the trailing DMA work is outs whose compute finished
long ago -- DMA engines run 100% busy from ~2.3us to ~75.4us. The tail
stage (den/recip/outMM/cast/outDMA) lags one period so PE never stalls on
exp; psum: 2 sim banks + 6-deep shared out/den ring. A dummy early matmul
warms the cost model's PE p-state ramp so setup MMs run at full clock.
"""

from collections import deque

import numpy as np
import ml_dtypes

import concourse.bacc as bacc
import concourse.mybir as mybir
import concourse.tile as tile
from concourse import bass_utils

F32 = mybir.dt.float32
F32R = mybir.dt.float32r
F16 = mybir.dt.float16
F8 = mybir.dt.float8e3

B, C, H, W = 8, 512, 128, 128
N = H * W                    # 16384 pixels per image
CK, CV, K = 256, 256, 64
P = 128                      # SBUF partitions
FP = 512                     # pixels per period (DMA granule)
NT = 4                       # sub-tiles per period
FT = FP // NT                # 128 pixels per sub-tile (= out PSUM partitions)
NP = N // FP                 # 32 periods
CI_CH = C // P               # 4 contraction chunks over C
Q_CH = CK // P               # 2 chunks over Ck
V_CH = CV // P               # 2 chunks over Cv
SCALE = CK ** -0.5           # 1/16
PF = 24                      # x prefetch run-ahead (SP queue depth paces actual issue)

N8 = 4                       # trailing C-chunks of x stored as fp8 E3M4
C16 = CI_CH - N8

_CACHED = None


def _build(n8):
    c16 = CI_CH - n8
    nc = bacc.Bacc("TRN2", target_bir_lowering=False, debug=False)

    X16 = (
        nc.dram_tensor("x16", [c16 * P, N], F16, kind="ExternalInput").ap()
        if c16 else None
    )
    X8 = (
        nc.dram_tensor("x8", [n8 * P, N], F8, kind="ExternalInput").ap()
        if n8 else None
    )
    # host-folded weight products (weights only, no input data):
    # wqk[c', c] = (Wk^T Wq)[c', c];  wovT[c', c] = (Wo Wv)^T[c', c]
    PF_T = nc.dram_tensor("pf16", [C, K], F16, kind="ExternalInput").ap()
    WQK = nc.dram_tensor("wqk", [C, C], F16, kind="ExternalInput").ap()
    WOVT = nc.dram_tensor("wovT", [C, C], F16, kind="ExternalInput").ap()
    # crow = [ones(64) | wobvo = Wo@bv+bo (512) | wqbk = Wq^T bk (512)]
    # (the SCALE*bq.bk logit constant is k-independent, softmax cancels it)
    CROW = nc.dram_tensor("crow", [1, 1088], F32, kind="ExternalInput").ap()
    # wkbq[c'] = Wk^T @ (bq*SCALE), chunked per-partition
    WKBQ = nc.dram_tensor("wkbq", [P, CI_CH], F16, kind="ExternalInput").ap()
    OUTT = nc.dram_tensor("outt", [N, C], F16, kind="ExternalOutput").ap()

    x16_r = X16.rearrange("(co p) n -> p co n", p=P) if c16 else None
    x8_r = X8.rearrange("(co p) n -> p co n", p=P) if n8 else None
    outt_r = OUTT.rearrange("(q t p) c -> p q t c", t=NT, p=P)

    Exp = mybir.ActivationFunctionType.Exp
    Ident = mybir.ActivationFunctionType.Identity

    with tile.TileContext(nc) as tc:
        with tc.tile_pool(name="const", bufs=1) as cp:
            pf_t = cp.tile([P, CI_CH, K], F16)
            nc.sync.dma_start(pf_t, PF_T.rearrange("(co p) k -> p co k", p=P))
            wqk = cp.tile([P, CI_CH, C], F16)
            nc.sync.dma_start(wqk, WQK.rearrange("(co p) c -> p co c", p=P))
            wovt = cp.tile([P, CI_CH, C], F16)
            # Pool SWDGE: lands concurrently without delaying x on SP/HWDGE
            nc.gpsimd.dma_start(wovt, WOVT.rearrange("(co p) c -> p co c", p=P))
            crow = cp.tile([1, 1088], F32R)
            nc.scalar.dma_start(crow, CROW.bitcast(F32R))
            ones_row = crow[:, 0:K]
            wobvo_row = crow[:, K:K + C]
            wqbk_row = crow[:, K + C:K + C + C]
            wkbq = cp.tile([P, CI_CH], F16)
            nc.scalar.dma_start(wkbq, WKBQ)
            ones_col = cp.tile([K, 1], F16)
            nc.vector.memset(ones_col, 1.0)

            wvt = cp.tile([K, C], F16)           # (Wo @ value^T)^T + bo
            msim = cp.tile([P, CI_CH, K], F16)   # M[c,k] = (Wq^T keym)[c,k]
            sbias = cp.tile([K, 1], F32)         # (bq/16)^T keym

            # ---- one-time, all straight from pf via host-folded weights
            with tc.tile_pool(name="setup_ps", bufs=1, space="PSUM") as sps:
                # dummy matmul as early as possible: the cost model's PE
                # p-state ramp counts from the first continuous-busy start,
                # so warming here gets setup MMs toward full clock early
                warm = sps.tile([1, 1], F32)
                nc.tensor.matmul(warm, ones_col, ones_col, start=True, stop=True)
                # msim[c,k] = sum_c' wqk[c',c] pf[c',k] + (Wq^T bk)[c] 1[k]
                mps = sps.tile([P, CI_CH, K], F32)
                for ci in range(CI_CH):
                    for cj in range(CI_CH):
                        nc.tensor.matmul(
                            mps[:, ci, :],
                            wqk[:, cj, ci * P:(ci + 1) * P],
                            pf_t[:, cj, :],
                            start=(cj == 0), stop=False,
                        )
                    nc.tensor.matmul(
                        mps[:, ci, :],
                        wqbk_row[:, ci * P:(ci + 1) * P],
                        ones_row,
                        start=False, stop=True,
                    )
                nc.vector.tensor_copy(msim, mps)

                sbps = sps.tile([K, 1], F32)
                for cj in range(CI_CH):
                    nc.tensor.matmul(
                        sbps, pf_t[:, cj, :], wkbq[:, cj:cj + 1],
                        start=(cj == 0), stop=(cj == CI_CH - 1),
                    )
                nc.vector.tensor_copy(sbias, sbps)

                wvtps = sps.tile([K, C], F32)
                for cj in range(CI_CH):
                    nc.tensor.matmul(
                        wvtps, pf_t[:, cj, :], wovt[:, cj, :],
                        start=(cj == 0), stop=False,
                    )
                # += ones[k] * (Wo bv + bo)[c]: softmax columns sum to 1, so
                # a constant row realises both the value bias and +bo.
                nc.tensor.matmul(
                    wvtps, ones_row, wobvo_row,
                    start=False, stop=True,
                )
                nc.vector.tensor_copy(wvt, wvtps)

            # ---- steady-state software pipeline over 512-px periods
            with (
                tc.tile_pool(name="xin16", bufs=(32 if c16 <= 2 else 26)) as xp16,
                tc.tile_pool(name="xin8", bufs=32) as xp8,
                tc.tile_pool(name="esb", bufs=8) as ep,
                tc.tile_pool(name="rsb", bufs=8) as rp,
                tc.tile_pool(name="outsb", bufs=16) as outp,
                tc.tile_pool(name="simps", bufs=2, space="PSUM") as simps,
                tc.tile_pool(name="outps", bufs=6, space="PSUM") as outps,
            ):
                xq = deque()

                def dispatch_x(j):
                    if j >= NP:
                        return
                    n0 = j * FP
                    t16 = t8 = None
                    if c16:
                        t16 = xp16.tile([P, c16, FP], F16, tag="x16")
                        nc.sync.dma_start(t16, x16_r[:, :, n0:n0 + FP])
                    if n8:
                        t8 = xp8.tile([P, n8, FP], F8, tag="x8")
                        nc.sync.dma_start(t8, x8_r[:, :, n0:n0 + FP])
                    xq.append((t16, t8))

                def tail(e_t, j):
                    den = outps.tile([P, NT], F32, tag="ops")
                    for t in range(NT):
                        nc.tensor.matmul(
                            den[:, t:t + 1], e_t[:, t * FT:(t + 1) * FT], ones_col,
                            start=True, stop=True,
                        )
                    r = rp.tile([P, NT], F32, tag="r")
                    with nc.allow_low_precision(reason="positive softmax denom"):
                        nc.vector.reciprocal(r, den)
                    o_sb = outp.tile([P, NT, C], F16, tag="osb")
                    for t in range(NT):
                        ops = outps.tile([P, C], F32, tag="ops")
                        nc.tensor.matmul(
                            ops, e_t[:, t * FT:(t + 1) * FT], wvt, start=True, stop=True,
                        )
                        sc = r[:, t:t + 1]
                        # alternate 2/2 and 1/3 ACT/DVE cast split so neither
                        # engine's steady load exceeds ~70% of the DMA period
                        n_act = 2 if (j % 3 != 2) else 1
                        if t < n_act:
                            nc.scalar.activation(o_sb[:, t, :], ops, Ident, scale=sc)
                        else:
                            nc.vector.tensor_scalar_mul(o_sb[:, t, :], ops, sc)
                    # out rides the Pool SWDGE queue: its wait-on-casts must
                    # not block SP's run-ahead x prefetch dispatches. In the
                    # drain (x stream done) SP is free and HWDGE has lower
                    # dispatch latency, so the last outs go there.
                    if j >= NP - 3:
                        nc.sync.dma_start(outt_r[:, j, :, :], o_sb)
                    else:
                        nc.gpsimd.dma_start(outt_r[:, j, :, :], o_sb)

                for j in range(PF):
                    dispatch_x(j)
                pend = deque()
                for ip in range(NP):
                    x16_t, x8_t = xq.popleft()
                    # full-width sim MMs: 4 instructions/period (PE.SEQ
                    # dispatch cost is per-instruction, engine time is not)
                    sim = simps.tile([K, FP], F32, tag="sim")
                    for ci in range(CI_CH):
                        if ci < c16:
                            src = x16_t[:, ci, :]
                        else:
                            src = x8_t[:, ci - c16, :]
                        nc.tensor.matmul(
                            sim, msim[:, ci, :], src,
                            start=(ci == 0), stop=(ci == CI_CH - 1),
                        )
                    e_t = ep.tile([K, FP], F16, tag="e")
                    nc.scalar.activation(e_t, sim, Exp, scale=SCALE, bias=sbias)
                    dispatch_x(ip + PF)
                    if pend:
                        tail(*pend.popleft())
                    pend.append((e_t, ip))
                while pend:
                    tail(*pend.popleft())

    nc.compile()
    return nc


def _get_nc():
    global _CACHED
    if _CACHED is None:
        _CACHED = _build(N8)
    return _CACHED


def kernel(x, proxy, Wq, bq, Wk, bk, Wv, bv, Wo, bo, **run_kwargs):
    nc = _get_nc()

    f32 = np.float32
    Wqf, Wkf, Wvf, Wof = (np.asarray(a, f32) for a in (Wq, Wk, Wv, Wo))
    bqf, bkf, bvf, bof = (np.asarray(a, f32) for a in (bq, bk, bv, bo))
    wqk = (Wkf.astype(np.float64).T @ Wqf.astype(np.float64)).astype(f32)
    wovT = (Wof.astype(np.float64) @ Wvf.astype(np.float64)).T.astype(f32)
    wobvo = Wof @ bvf + bof
    wqbk = Wqf.T @ bkf
    wkbq = (Wkf.T @ (bqf * np.float32(SCALE))).reshape(CI_CH, P).T
    crow = np.concatenate(
        [np.ones((1, K), f32), wobvo.reshape(1, C),
         wqbk.reshape(1, C)], axis=1)
    shared = {
        "wqk": np.ascontiguousarray(wqk).astype(np.float16),
        "wovT": np.ascontiguousarray(wovT).astype(np.float16),
        "wkbq": np.ascontiguousarray(wkbq).astype(np.float16),
        "crow": np.ascontiguousarray(crow),
    }
    in_maps = []
    for b in range(B):
        m = dict(shared)
        xf = np.asarray(x[b]).reshape(C, N)
        if C16:
            m["x16"] = np.ascontiguousarray(xf[: C16 * P]).astype(np.float16)
        if N8:
            m["x8"] = np.ascontiguousarray(xf[C16 * P:]).astype(
                ml_dtypes.float8_e3m4
            )
        m["pf16"] = np.ascontiguousarray(
            np.asarray(proxy[b, :, :, 0])
        ).astype(np.float16)
        in_maps.append(m)
    res = bass_utils.run_bass_kernel_spmd(
        nc, in_maps, core_ids=list(range(B)), **run_kwargs
    )
    out = np.stack(
        [
            np.asarray(res.results[b]["outt"]).astype(np.float32).T
            for b in range(B)
        ],
        axis=0,
    )
    if run_kwargs:
        kernel.last_results = res
    return out.reshape(B, C, H, W)
